# revision 38
# baseline (speedup 1.0000x reference)
"""MDTA Bass kernel for 8 TRN2 NeuronCores, two SPMD launches.

Math (row-major reshape of the reference):
  q.reshape(B,HEADS,HW,D) maps tensor[b,hd,s,d] = conv[b,16hd+ci,y,16xs+d],
  s = ci*1024 + y*8 + xs, so the attention feature axis d is x%16 and
  G[hd,d,j] = sum_{ci,y,xs} k2[16hd+ci,y,16xs+d] * q2[16hd+ci,y,16xs+j]
  out_conv[b,16hd+j, ci*8+y//16, (y%16)*8+xs] = sum_d v2[16hd+ci,y,16xs+d]*P[d,j]

Key structural choices vs a naive port:
  * conv1x1+conv3x3 compose into ONE 3x3 conv (both linear): w2' = w2 @ w1
    precomputed on host; the device runs a single 9-tap matmul chain per
    q/k/v. The wo 1x1 conv folds into the attention-apply matmul the same
    way (M = PSTACK @ wo^T on host); LayerNorm stats (rstd, -mu*rstd) are
    host-precomputed; the residual add happens on host.
  * Everything runs in fp16 (default SCORE="f16t"): fp16's 10-bit mantissa
    matches the PE's internal f32r rounding, runs 1 cyc/row at any output
    width, and (unlike bf16, which fails the 2e-2 gate - softmax amplifies
    8-bit-mantissa logit errors ~10x) keeps the gram matrix accurate.
  * The q/k convs run TRANSPOSED (stationary = xn row windows, moving =
    weights): each output row lands in PSUM as [x, channel], which IS the
    level-1-transposed layout - no separate transpose pass. A strided
    PSUM->SBUF copy groups it as tb[x, (hd,oct,y8,ci)]; one fp16 PE
    transpose level then yields the [(y8,ci), x] pair operands.
  * fp8 DoubleRow conv and DMA-XBAR (SBUF->SBUF) transposes were tried and
    rejected: fp8 costs 2e-2 rel err; the XBAR transpose returns corrupt
    data on real hardware despite simulating correctly.
  * PSUM->SBUF copies alternate DVE/Act (GPSIMD cannot touch PSUM); the
    Pool engine takes normalize rows instead.

Launch 1 (spatial shards: b x quarter-of-H, 1-row halo): normalize, three
9-tap fp16 convs (q/k transposed), per-head transposes + 128x128 pair
matmuls. Outputs v2 (fp16) + pairs (f32). Host: strip-diagonal gram,
softmax, M = PSTACK @ wo^T, VROW shuffle of v2 (pure byte movement).
Launch 2: 32 matmuls [128,128] (M^T @ VROW = wo(attention) rows, fp16).
Host: upcast + residual.
"""

import os
from contextlib import ExitStack

import numpy as np

import concourse.bacc as bacc
import concourse.bass as bass
import concourse.mybir as mybir
import concourse.tile as tile
from concourse import bass_utils

F32 = mybir.dt.float32
F32R = mybir.dt.float32r
BF16 = mybir.dt.bfloat16
F16 = mybir.dt.float16
F8 = mybir.dt.float8e4
AX = mybir.AxisListType
ALU = mybir.AluOpType
ACT = mybir.ActivationFunctionType

NPF16 = np.float16

B, C, H, W = 2, 128, 128, 128
HEADS, D = 8, 16
EPS = 1e-5
RPC = H // 4          # output rows per core
RH = RPC + 2          # with 1-row halo each side
NPIX = RPC * W        # 4096
NHAL = RH * W         # 4352
WP = W + 2            # padded width

_CACHE = {}

# score-path config for q/k transposes + pair matmuls:
#   "f32r":   PE transposes f32r (1.5 cyc/row), pairs f32r (4 cyc/row)
#   "f16":    fp16 end-to-end, T2 on the DMA XBAR (fp16 PSUM + DMA-T)
#   "f16pe":  fp16 end-to-end, T2 on the PE (tests fp16 PSUM without DMA-T)
#   "f16dma": c2 fp32, T1 f32r on PE, tb fp16, T2 on the DMA XBAR
#             (tests DMA-T without fp16 PSUM)
#   "bf16":   fails the 2e-2 gate (softmax amplifies 8-bit mantissa logits)
SCORE = os.environ.get("KERNEL_SCORE", "f16t")
# (SD16, t1pe16, t2dma, trans)
# trans: all convs in fp16 with the q/k convs writing transposed [x, co]
# rows straight into the pair layout - no level-1 transposes at all
_SCORE_CFG = {
    "f32r": (F32, False, False, False),
    "f16": (F16, True, True, False),
    "f16pe": (F16, True, False, False),
    "f16t": (F16, True, False, True),
    "f16dma": (F16, False, True, False),
    "bf16": (BF16, True, True, False),
}
# v-path conv in fp8e4m3 with DoubleRow perf mode (2 taps per pass). The
# v path tolerates ~3% conv error (softmax-free smooth averaging).
V8 = os.environ.get("KERNEL_V8", "0") == "1"


def _build_l1(affine):
    SD16, t1pe16, t2dma, trans = _SCORE_CFG[SCORE]
    c2d = SD16 if t1pe16 else F32      # q/k conv-output storage
    tbd = SD16                          # tb / t4 / pair-input storage
    idnd = SD16 if t1pe16 else F32R
    cvd = F16 if trans else F32R       # conv operand dtype (weights + xn)
    xnd = F16 if trans else F32

    def r32(ap):
        # f32r matmuls require operands (and their producers) typed f32r
        return ap.bitcast(F32R) if ap.dtype == F32 else ap

    nc = bacc.Bacc("TRN2", target_bir_lowering=False, debug=False, num_devices=8)
    x_d = nc.dram_tensor("x_sl", [128, NHAL], F16,
                         kind="ExternalInput").ap()
    ln_d = nc.dram_tensor("ln", [128, 2 * RH], F32, kind="ExternalInput").ap()
    w_d = {t: nc.dram_tensor(f"w{t}", [128, 9 * 128], cvd,
                         kind="ExternalInput").ap()
           for t in ("qk" if V8 else "qkv")}
    if V8:
        w_d["v"] = nc.dram_tensor("wv", [128, 10 * 128], F8,
                                  kind="ExternalInput").ap()
    idn_d = nc.dram_tensor("ident", [128, 128], idnd,
                         kind="ExternalInput").ap()
    if affine:
        gm_d = nc.dram_tensor("gamma_b", [128, W], F32, kind="ExternalInput").ap()
        bt_d = nc.dram_tensor("beta_b", [128, W], F32, kind="ExternalInput").ap()
    v2_d = nc.dram_tensor("v2o", [128, NPIX], F16, kind="ExternalOutput").ap()
    pr_d = nc.dram_tensor("pairs", [128, 8 * 128], F32, kind="ExternalOutput").ap()

    with tile.TileContext(nc) as tc, ExitStack() as ctx:
        consts = ctx.enter_context(tc.tile_pool(name="consts", bufs=1))
        big = ctx.enter_context(tc.tile_pool(name="big", bufs=1))
        sbw = ctx.enter_context(tc.tile_pool(name="sbw", bufs=3))
        ps_cv = ctx.enter_context(tc.tile_pool(name="ps_cv", bufs=2, space="PSUM"))
        ps_t1 = ctx.enter_context(tc.tile_pool(name="ps_t1", bufs=2, space="PSUM"))
        ps_t2 = ctx.enter_context(tc.tile_pool(name="ps_t2", bufs=2, space="PSUM"))
        ps_pr = ctx.enter_context(tc.tile_pool(name="ps_pr", bufs=2, space="PSUM"))

        # PSUM drains alternate DVE/Act (GPSIMD cannot access PSUM)
        cpe = [nc.vector.tensor_copy,
               lambda o, i: nc.scalar.copy(o, i)]
        cpi = [0]

        def cp(out, in_):
            cpe[cpi[0] % 2](out, in_)
            cpi[0] += 1

        # ---- hoist the Act func-table load off the critical path ----
        dmy = sbw.tile([128, 1], F32, name="dmy", tag="dmy", bufs=1)
        nc.vector.memset(dmy[:], 0.0)
        nc.scalar.copy(dmy[:], dmy[:])

        # ---- inputs: ln + first x chunk + q-weights lead; f32r rounding
        # happens inside the PE, so staged fp32 weights are bitcast directly
        LNC = 0
        x_t = big.tile([128, NHAL], F16, name="x_t", tag="x_t")
        ln = consts.tile([128, 2 * RH], F32, name="ln", tag="ln")
        chunks = ((0, 6), (6, 15), (15, 24), (24, 34))
        w = {}
        for t in "qkv":
            wd = (F8, 10 * 128) if (V8 and t == "v") else (cvd, 9 * 128)
            w[t] = consts.tile([128, wd[1]], wd[0], name=f"w{t}r", tag=f"w{t}r")
        nc.sync.dma_start(ln[:], ln_d[:])
        nc.sync.dma_start(x_t[:, 0:3 * W], x_d[:, 0:3 * W])
        nc.sync.dma_start(x_t[:, 3 * W:6 * W], x_d[:, 3 * W:6 * W])
        for i in range(3):
            nc.sync.dma_start(w["q"][:, i * 384:(i + 1) * 384],
                              w_d["q"][:, i * 384:(i + 1) * 384])
        for a, b_ in chunks[1:]:
            nc.sync.dma_start(x_t[:, a * W:b_ * W], x_d[:, a * W:b_ * W])
        for t in "kv":
            nc.sync.dma_start(w[t][:], w_d[t][:])
        idn = consts.tile([128, 128], idnd, name="idn", tag="idn")
        nc.sync.dma_start(idn[:], idn_d[:])
        if affine:
            gm = consts.tile([128, W], F32, name="gm", tag="gm")
            nc.sync.dma_start(gm[:], gm_d[:])
            bt = consts.tile([128, W], F32, name="bt", tag="bt")
            nc.sync.dma_start(bt[:], bt_d[:])

        # ---- normalize (host-computed rstd/nmr), pipelined per x-chunk ----
        xn = big.tile([128, RH * WP], xnd, name="xn", tag="xn")
        xnv = xn.rearrange("p (r w) -> p r w", w=WP)
        zpad = sbw.tile([128, RH], F32, name="zpad", tag="zpad", bufs=1)
        nc.vector.memset(zpad[:], 0.0)
        zp3 = zpad.rearrange("p (r o) -> p r o", o=1)
        nc.vector.tensor_copy(r32(xnv[:, :, 0:1]), zp3)
        nc.vector.tensor_copy(r32(xnv[:, :, WP - 1:WP]), zp3)
        for a, b_ in chunks:
            for r in range(a, b_):
                dst = xnv[:, r, 1:1 + W]
                src = x_t[:, LNC + r * W:LNC + (r + 1) * W]
                if r < 6:
                    nc.vector.tensor_scalar(r32(dst), src,
                                            ln[:, r:r + 1],
                                            ln[:, RH + r:RH + r + 1],
                                            op0=ALU.mult, op1=ALU.add)
                elif r % 3 == 0:
                    nc.scalar.activation(r32(dst), src, ACT.Identity,
                                         bias=ln[:, RH + r:RH + r + 1],
                                         scale=ln[:, r:r + 1])
                elif r % 3 == 1:
                    nc.gpsimd.tensor_scalar(r32(dst), src,
                                            ln[:, r:r + 1],
                                            ln[:, RH + r:RH + r + 1],
                                            op0=ALU.mult, op1=ALU.add)
                else:
                    nc.vector.tensor_scalar(r32(dst), src,
                                            ln[:, r:r + 1],
                                            ln[:, RH + r:RH + r + 1],
                                            op0=ALU.mult, op1=ALU.add)
                if affine:
                    nc.vector.tensor_tensor(r32(dst), dst, gm[:],
                                            op=ALU.mult)
                    nc.gpsimd.tensor_tensor(r32(dst), dst, bt[:],
                                            op=ALU.add)

        if V8:
            xn8 = big.tile([128, RH * WP], F8, name="xn8", tag="xn8")
            for a, b_ in chunks:
                nc.gpsimd.tensor_copy(xn8[:, a * WP:b_ * WP],
                                      xn[:, a * WP:b_ * WP])

        # ---- 9-tap convs (composed weights) ----
        c2 = {}

        def conv_group(t, g, sink):
            ps = ps_cv.tile([128, 512], F32, name=f"cv{t}{g}", tag="cv")
            if V8 and t == "v":
                # fp8 DoubleRow: two taps per pass (tap 8 pairs with zero
                # weights); custom APs pack the two shifted windows and the
                # two weight rows along a stride-Δ pair dim
                wv = w["v"][:]
                x8 = xn8[:]
                for i in range(4):
                    for pr5 in range(5):
                        o1 = 2 * pr5
                        o2 = min(o1 + 1, 8)
                        b1 = (o1 // 3) * WP + o1 % 3
                        delta = ((o2 // 3) * WP + o2 % 3) - b1
                        lhsT = bass.AP(wv.tensor, wv.offset + 256 * pr5,
                                       [list(wv.ap)[0], [128, 2], [1, 128]])
                        rhs = bass.AP(x8.tensor,
                                      x8.offset + (4 * g + i) * WP + b1,
                                      [list(x8.ap)[0], [delta, 2], [1, 128]])
                        nc.tensor.matmul(ps[:, i * 128:(i + 1) * 128], lhsT,
                                         rhs, start=(pr5 == 0),
                                         stop=(pr5 == 4),
                                         perf_mode=mybir.MatmulPerfMode.DoubleRow)
            else:
                for off in range(9):
                    dy, dx = off // 3, off % 3
                    rhs = xnv[:, 4 * g + dy:4 * g + dy + 4, dx:dx + 128]
                    nc.tensor.matmul(ps[:], w[t][:, off * 128:(off + 1) * 128],
                                     r32(rhs), start=(off == 0),
                                     stop=(off == 8))
            sink(g, ps)

        def conv_group_trans(t, gq):
            # transposed q/k conv: stationary = xn windows, moving = weights;
            # each output row lands as [x, co] - the level-1 transposed layout
            tb5 = tbig[t].rearrange("p (h o y c) -> p h o y c", h=8, o=4, y=8)
            ps = ps_t1.tile([128, 512], F32, name=f"ct{t}{gq}", tag="t1")
            for i in range(4):
                y = 4 * gq + i
                for off in range(9):
                    dy, dx = off // 3, off % 3
                    lhsT = xnv[:, y + dy, dx:dx + 128]
                    nc.tensor.matmul(ps[:, i * 128:(i + 1) * 128], lhsT,
                                     w[t][:, off * 128:(off + 1) * 128],
                                     start=(off == 0), stop=(off == 8))
            o, ys = gq // 2, 4 * (gq % 2)
            src = ps.rearrange("p (r h c) -> p h r c", r=4, h=8)
            cp(r32(tb5[:, :, o, ys:ys + 4, :]), src)

        def qk_sink(t):
            dst = big.tile([128, NPIX], c2d, name=f"c2{t}", tag=f"c2{t}")
            c2[t] = dst

            def sink(g, ps):
                cp(r32(dst[:, g * 512:(g + 1) * 512]), ps[:])
            return sink

        def v_sink(g, ps):
            vst = sbw.tile([128, 512], F16, name=f"vst{g}", tag="vst", bufs=3)
            cp(vst[:], ps[:])
            nc.sync.dma_start(v2_d[:, g * 512:(g + 1) * 512], vst[:])

        # ---- level-1 transpose: tb[x, (hd,o,y8,ci)] = c2[16hd+ci, (8o+y8)*128+x]
        tbig = {}

        def t1_group(t, gq):
            tb5 = tbig[t].rearrange("p (h o y c) -> p h o y c", h=8, o=4, y=8)
            psT = ps_t1.tile([128, 512], c2d if t1pe16 else F32,
                             name=f"t1{t}{gq}", tag="t1")
            for i in range(4):
                y = 4 * gq + i
                nc.tensor.transpose(r32(psT[:, i * 128:(i + 1) * 128]),
                                    r32(c2[t][:, y * 128:(y + 1) * 128]),
                                    idn[:])
            o, ys = gq // 2, 4 * (gq % 2)
            src = psT.rearrange("p (r h c) -> p h r c", r=4, h=8)
            cp(r32(tb5[:, :, o, ys:ys + 4, :]), src)

        # ---- level-2 transpose + per-head pair matmul, interleaved with v ----
        pair_sb = big.tile([128, 8 * 128], F32, name="pair_sb", tag="pair_sb")
        t4 = {}
        pps = {}

        def t2_head(hd):
            for t in "qk":
                tb5 = tbig[t].rearrange("p (h f) -> p h f", h=8)
                sb = sbw.tile([128, 512], tbd, name=f"t4{t}{hd}", tag=f"t4{t}",
                              bufs=2)
                if t2dma:
                    # 16-bit transpose runs on the DMA engines' XBAR: no PE,
                    # no PSUM, no drain copy
                    for o in range(4):
                        nc.sync.dma_start(sb[:, o * 128:(o + 1) * 128],
                                          tb5[:, hd, o * 128:(o + 1) * 128],
                                          transpose=True)
                else:
                    psT2 = ps_t2.tile([128, 512], tbd if t1pe16 else F32,
                                      name=f"t2{t}{hd}", tag="t2")
                    for o in range(4):
                        nc.tensor.transpose(r32(psT2[:, o * 128:(o + 1) * 128]),
                                            r32(tb5[:, hd, o * 128:(o + 1) * 128]),
                                            idn[:])
                    cp(r32(sb[:]), psT2[:])
                t4[t, hd] = sb

        def pair_head(hd):
            pp = ps_pr.tile([128, 128], F32, name=f"pps{hd}", tag="pps")
            for o in range(4):
                nc.tensor.matmul(pp[:], r32(t4["k", hd][:, o * 128:(o + 1) * 128]),
                                 r32(t4["q", hd][:, o * 128:(o + 1) * 128]),
                                 start=(o == 0), stop=(o == 3))
            cp(pair_sb[:, hd * 128:(hd + 1) * 128], pp[:])
            nc.sync.dma_start(pr_d[:, hd * 128:(hd + 1) * 128],
                              pair_sb[:, hd * 128:(hd + 1) * 128])

        qs = qk_sink("q")
        tbig["q"] = big.tile([128, NPIX], tbd, name="tbq", tag="tbq")
        ks = qk_sink("k")
        tbig["k"] = big.tile([128, NPIX], tbd, name="tbk", tag="tbk")
        if trans:
            for g in range(8):
                conv_group_trans("q", g)
            for g in range(8):
                conv_group_trans("k", g)
        else:
            for g in range(8):
                conv_group("q", g, qs)
            for g in range(8):
                t1_group("q", g)
            for g in range(8):
                conv_group("k", g, ks)
            for g in range(8):
                t1_group("k", g)
        # v groups interleaved with per-head transpose+pair blocks: the v conv
        # keeps the PE busy while the pair operands drain from PSUM, and the
        # last pair's copy/DMA tail hides under the last v group.
        t2_head(0)
        t2_head(1)
        for i in range(8):
            pair_head(i)
            conv_group("v", i, v_sink)
            if i < 6:
                t2_head(i + 2)

    nc.compile()
    return nc


def _build_l2():
    nc = bacc.Bacc("TRN2", target_bir_lowering=False, debug=False, num_devices=8)
    m_d = nc.dram_tensor("mT", [128, 128], F16, kind="ExternalInput").ap()
    vr_d = nc.dram_tensor("vrows", [128, NPIX], F16, kind="ExternalInput").ap()
    y_d = nc.dram_tensor("y_att", [128, NPIX], F16, kind="ExternalOutput").ap()

    with tile.TileContext(nc) as tc, ExitStack() as ctx:
        consts = ctx.enter_context(tc.tile_pool(name="consts", bufs=1))
        big = ctx.enter_context(tc.tile_pool(name="big", bufs=1))
        sbw = ctx.enter_context(tc.tile_pool(name="sbw", bufs=3))
        ps_a = ctx.enter_context(tc.tile_pool(name="ps_a", bufs=2, space="PSUM"))

        cpe = [nc.vector.tensor_copy,
               lambda o, i: nc.scalar.copy(o, i),
               nc.gpsimd.tensor_copy]
        cpi = [0]

        vr = big.tile([128, NPIX], F16, name="vr", tag="vr")
        mT = consts.tile([128, 128], F16, name="mT", tag="mT")
        # mT first (gates the weight load), then ascending vr chunks: a small
        # first chunk starts the matmuls early, bigger later chunks amortize
        # the per-DMA latency
        nc.sync.dma_start(mT[:], m_d[:])
        nc.sync.dma_start(vr[:, 0:256], vr_d[:, 0:256])
        for a, b_ in ((256, 1280), (1280, 2560), (2560, 4096)):
            nc.sync.dma_start(vr[:, a:b_], vr_d[:, a:b_])

        yst = big.tile([128, NPIX], F16, name="yst", tag="yst")
        for g in range(8):
            ps = ps_a.tile([128, 512], F32, name=f"ar{g}", tag="ar")
            for i in range(4):
                y = 4 * g + i
                nc.tensor.matmul(ps[:, i * 128:(i + 1) * 128], mT[:],
                                 vr[:, y * 128:(y + 1) * 128],
                                 start=True, stop=True)
            seg = slice(g * 512, (g + 1) * 512)
            cpe[cpi[0] % 2](yst[:, seg], ps[:])
            cpi[0] += 1
            nc.sync.dma_start(y_d[:, seg], yst[:, seg])

    nc.compile()
    return nc


def _get(name, affine=False):
    key = (name, affine, SCORE, V8)
    if key not in _CACHE:
        _CACHE[key] = _build_l1(affine) if name == "l1" else _build_l2()
    return _CACHE[key]


def _host_middle(pairs_list, scale, wo2d):
    """pairs -> gram -> softmax P -> M = PSTACK @ wo^T per batch (bf16)."""
    f = np.float32
    G = np.zeros((B, HEADS, D, D), f)
    for c in range(8):
        pr = pairs_list[c].reshape(128, 8, 128)
        for hd in range(HEADS):
            blk = pr[:, hd, :].reshape(8, 16, 8, 16)      # [xs, d, xs', j]
            G[c // 4, hd] += np.einsum("adaj->dj", blk)
    G /= float(np.asarray(scale, f)[0])
    Gm = G - G.max(-1, keepdims=True)
    E = np.exp(Gm)
    P = (E / E.sum(-1, keepdims=True)).astype(f)          # [B, HEADS, 16, 16]

    ms = []
    for b in range(B):
        pstack = np.zeros((128, 128), f)
        for hd in range(HEADS):
            pstack[16 * hd:16 * hd + 16, 16 * hd:16 * hd + 16] = P[b, hd]
        ms.append(np.ascontiguousarray(pstack @ wo2d.T).astype(NPF16))
    return ms


def _vrows(v2o_list):
    """v2 (bf16, conv layout) -> per-core VROW matrices (pure indexing)."""
    v_conv = np.empty((B, C, H, W), NPF16)
    for c in range(8):
        b, r0 = c // 4, 32 * (c % 4)
        v_conv[b, :, r0:r0 + RPC, :] = v2o_list[c].reshape(C, RPC, W)
    # vc[b, hd, ci, y, xs, d]
    vc = v_conv.reshape(B, HEADS, 16, H, 8, 16)
    vrows = []
    for c in range(8):
        b, r0 = c // 4, 32 * (c % 4)
        rows = np.empty((32, 128, 128), NPF16)
        for i in range(32):
            yp = r0 + i
            ci, yb = yp // 8, yp % 8
            blk = vc[b, :, ci, 16 * yb:16 * yb + 16, :, :]   # [hd, yy, xs, d]
            rows[i] = blk.transpose(0, 3, 1, 2).reshape(128, 128)
        vrows.append(np.ascontiguousarray(rows.transpose(1, 0, 2))
                     .reshape(128, NPIX))
    return vrows


def _maps_l1(x, gamma, beta, wq1, wq2, wk1, wk2, wv1, wv2, affine):
    f = np.float32
    xp = np.pad(np.asarray(x, f), ((0, 0), (0, 0), (1, 1), (0, 0)))
    # host-side LayerNorm stats over the padded rows (pad rows: mu=0, xn=0)
    mu = xp.mean(-1)
    var = xp.var(-1)
    rstd = (1.0 / np.sqrt(var + EPS)).astype(f)
    nmr = (-mu * rstd).astype(f)
    idnd = _SCORE_CFG[SCORE][0] if _SCORE_CFG[SCORE][1] else F32R
    common = {"ident": np.eye(128, dtype=mybir.dt.np(idnd))}
    if affine:
        common["gamma_b"] = np.broadcast_to(np.asarray(gamma, f), (128, W)).copy()
        common["beta_b"] = np.broadcast_to(np.asarray(beta, f), (128, W)).copy()
    for t, w1_, w2_ in (("q", wq1, wq2), ("k", wk1, wk2), ("v", wv1, wv2)):
        w1n = np.asarray(w1_, f)[:, :, 0, 0]               # [cm, ci]
        w2n = np.asarray(w2_, f)                           # [co, cm, 3, 3]
        w2p = np.einsum("omyx,mi->oiyx", w2n, w1n)         # composed 3x3
        wt = w2p.transpose(1, 2, 3, 0).reshape(128, 9, 128)
        if V8 and t == "v":
            # tap 8 pairs with a zero tap; pair p holds taps (2p, 2p+1)
            wt = np.concatenate([wt, np.zeros((128, 1, 128), f)], axis=1)
            wt[:, 9] = wt[:, 8]
            wt[:, 8] = 0.0
            # executor layout [K, 2, M]: pair p at cols [256p, 256p+256) is
            # (zero, tap8) for p=4 -> swap so tap8 is slot0 with zero slot1
            wt[:, 8], wt[:, 9] = wt[:, 9].copy(), wt[:, 8].copy()
            common[f"w{t}"] = np.ascontiguousarray(
                wt.reshape(128, 10 * 128)).astype(mybir.dt.np(F8))
        else:
            wnp = wt.reshape(128, 9 * 128)
            if _SCORE_CFG[SCORE][3]:
                wnp = wnp.astype(np.float16)
            common[f"w{t}"] = np.ascontiguousarray(wnp)
    maps = []
    for c in range(8):
        b, r0 = c // 4, 32 * (c % 4)
        m = dict(common)
        m["x_sl"] = np.ascontiguousarray(
            xp[b, :, r0:r0 + RH, :].reshape(128, NHAL).astype(NPF16))
        m["ln"] = np.ascontiguousarray(
            np.concatenate([rstd[b, :, r0:r0 + RH], nmr[b, :, r0:r0 + RH]],
                           axis=1))
        maps.append(m)
    return maps


def _run(nc, maps, key):
    trace = bool(int(os.environ.get("KERNEL_TRACE", "0")))
    if _CACHE.get("sim"):
        from concourse.bass_interp import MultiCoreSim
        sim = MultiCoreSim(nc, num_cores=8, require_finite=True, require_nnan=True)
        cores = list(sim.cores.values())
        for c, m in enumerate(maps):
            for k, v in m.items():
                cores[c].tensor(k)[:] = v
        sim.simulate(check_with_hw=False)
        return [{k: np.array(cores[c].tensor(k)) for k in key} for c in range(8)]
    res = bass_utils.run_bass_kernel_spmd(nc, maps, core_ids=list(range(8)),
                                          trace=trace)
    _CACHE.setdefault("results", []).append(res)
    return res.results


def kernel(x, gamma, beta, scale, wq1, wq2, wk1, wk2, wv1, wv2, wo):
    f = np.float32
    affine = not (np.all(np.asarray(gamma, f) == 1.0)
                  and np.all(np.asarray(beta, f) == 0.0))
    r1 = _run(_get("l1", affine),
              _maps_l1(x, gamma, beta, wq1, wq2, wk1, wk2, wv1, wv2, affine),
              ("v2o", "pairs"))
    wo2d = np.asarray(wo, f)[:, :, 0, 0]
    ms = _host_middle([r["pairs"] for r in r1], scale, wo2d)
    vrows = _vrows([r["v2o"] for r in r1])
    maps2 = [{"mT": ms[c // 4], "vrows": vrows[c]} for c in range(8)]
    r2 = _run(_get("l2"), maps2, ("y_att",))
    y = np.empty((B, C, H, W), f)
    for c in range(8):
        b, r0 = c // 4, 32 * (c % 4)
        y[b, :, r0:r0 + RPC, :] = r2[c]["y_att"].astype(f).reshape(C, RPC, W)
    return y + np.asarray(x, f)


def kernel_sim(**inputs):
    _CACHE["sim"] = True
    try:
        return kernel(**inputs)
    finally:
        _CACHE["sim"] = False
